# revision 1
# baseline (speedup 1.0000x reference)
import sys

if '/opt/trn_rl_repo' not in sys.path:
    sys.path.insert(0, '/opt/trn_rl_repo')

import numpy as np

B, D, Q, H = 16, 1024, 128, 1024
NCORES = 8
NB = B // NCORES
NT = D // 128
HHALF = 512

_CACHE = {}


def build_nc(repeats=1, skip=(), xpose="pe", c4="gp"):
    import concourse.bacc as bacc
    import concourse.tile as tile
    from concourse import mybir, masks
    import concourse.bass as bass
    from contextlib import ExitStack

    ts = bass.ts
    f32 = mybir.dt.float32
    bf16 = mybir.dt.bfloat16
    i32 = mybir.dt.int32
    AF = mybir.ActivationFunctionType
    ALU = mybir.AluOpType

    nc = bacc.Bacc("TRN2", target_bir_lowering=False, debug=False)

    Ud_dram = nc.dram_tensor("U_d", [NB, D, H], f32, kind="ExternalInput")
    Uq_dram = nc.dram_tensor("U_q", [NB, Q, H], f32, kind="ExternalInput")
    w_dram = nc.dram_tensor("wc_w", [128, 3, NT], f32, kind="ExternalInput")
    qb_dram = nc.dram_tensor("q_mask", [NB, 128, 1], f32, kind="ExternalInput")
    db_dram = nc.dram_tensor("d_mask", [NB, 128, NT], f32, kind="ExternalInput")
    V_dram = nc.dram_tensor("V", [NB, D, 4 * H], f32, kind="ExternalOutput")

    with tile.TileContext(nc) as tc, ExitStack() as ctx:
        const = ctx.enter_context(tc.tile_pool(name="const", bufs=1))
        big = ctx.enter_context(tc.tile_pool(name="big", bufs=2))
        med = ctx.enter_context(tc.tile_pool(name="med", bufs=2))
        vec = ctx.enter_context(tc.tile_pool(name="vec", bufs=2))
        outp = ctx.enter_context(tc.tile_pool(name="outp", bufs=4))
        utp = ctx.enter_context(tc.tile_pool(name="utp", bufs=1))
        ps_big = ctx.enter_context(tc.tile_pool(name="ps_big", bufs=1, space="PSUM"))
        ps_mm = ctx.enter_context(tc.tile_pool(name="ps_mm", bufs=3, space="PSUM"))
        ps_sm = ctx.enter_context(tc.tile_pool(name="ps_sm", bufs=3, space="PSUM"))

        w_cols = const.tile([128, 3, NT], f32, tag="wcols")
        nc.gpsimd.dma_start(w_cols[:], w_dram[:])
        wd16 = const.tile([128, NT], bf16, tag="wd16")
        wq16 = const.tile([128, NT], bf16, tag="wq16")
        nc.vector.tensor_copy(wd16[:], w_cols[:, 0, :])
        nc.vector.tensor_copy(wq16[:], w_cols[:, 1, :])
        ident16 = const.tile([128, 128], bf16, tag="id16")
        masks.make_identity(nc, ident16[:])
        ident1f = const.tile([1, 1], f32, tag="id1f")
        nc.vector.memset(ident1f[:], 1.0)
        identf = const.tile([128, 128], f32, tag="idf")
        masks.make_identity(nc, identf[:])

        batch_seq = [bb for _ in range(repeats) for bb in range(NB)]
        preloaded = {}
        for i, b in enumerate(batch_seq[:2]):
            Ud = big.tile([128, NT, H], f32, tag="Ud")
            Ud_src = Ud_dram[b].rearrange("(t p) h -> p t h", p=128)
            for t in range(NT):
                if i == 0 and t == 0:
                    for q4 in range(4):
                        nc.sync.dma_start(Ud[:, t, ts(q4, 256)],
                                          Ud_src[:, t, ts(q4, 256)])
                else:
                    nc.sync.dma_start(Ud[:, t, :], Ud_src[:, t, :])
            Uq16 = med.tile([128, H], bf16, tag="Uq16")
            nc.gpsimd.dma_start(Uq16[:], Uq_dram[b])
            qbias = vec.tile([128, 1], f32, tag="qbias")
            nc.sync.dma_start(qbias[:], qb_dram[b])
            dbias = vec.tile([128, NT], f32, tag="dbias")
            nc.sync.dma_start(dbias[:], db_dram[b])
            preloaded[i] = (Ud, Uq16, qbias, dbias)

        for bi, b in enumerate(batch_seq):
            if bi in preloaded:
                Ud, Uq16, qbias, dbias = preloaded[bi]
            else:
                Ud = big.tile([128, NT, H], f32, tag="Ud")
                Ud_src = Ud_dram[b].rearrange("(t p) h -> p t h", p=128)
                for t in range(NT):
                    nc.sync.dma_start(Ud[:, t, :], Ud_src[:, t, :])
                Uq16 = med.tile([128, H], bf16, tag="Uq16")
                nc.gpsimd.dma_start(Uq16[:], Uq_dram[b])
                qbias = vec.tile([128, 1], f32, tag="qbias")
                nc.sync.dma_start(qbias[:], qb_dram[b])
                dbias = vec.tile([128, NT], f32, tag="dbias")
                nc.sync.dma_start(dbias[:], db_dram[b])

            def emit_ud_copy_out(ts_=range(NT), b_=None, Ud_=None):
                if "out_dma" in skip:
                    return
                b2 = b if b_ is None else b_
                U2 = Ud if Ud_ is None else Ud_
                for t in ts_:
                    nc.sync.dma_start(
                        V_dram[b2, t * 128:(t + 1) * 128, 0:H], U2[:, t, :])
            if bi % NB == 0:
                emit_ud_copy_out(range(4))
            UdT = big.tile([128, NT, D], bf16, tag="UdT")
            UqT = med.tile([128, NT, Q], bf16, tag="UqT")
            if xpose == "xbar":
                Ud16 = big.tile([128, NT, H], bf16, tag="Ud16")
                for t in range(NT):
                    nc.vector.tensor_copy(Ud16[:, t, :], Ud[:, t, :])
                for t in range(NT):
                    nc.sync.dma_start_transpose(UdT[:, :, ts(t, 128)], Ud16[:, t, :])
                nc.sync.dma_start_transpose(UqT[:], Uq16[:])
            else:
                for t in range(NT):
                    for k in range(NT):
                        pool_ = ps_sm
                        tp = pool_.tile([128, 128], f32, tag="psm")
                        nc.tensor.transpose(tp[:], Ud[:, t, ts(k, 128)], identf[:])
                        ev = nc.scalar.copy if (k % 2 == 0) else (
                            lambda o, i: nc.vector.tensor_copy(o, i))
                        ev(UdT[:, k, ts(t, 128)], tp[:])
                for k in range(NT):
                    tq = ps_sm.tile([128, Q], bf16, tag="psm")
                    nc.tensor.transpose(tq[:], Uq16[:, ts(k, 128)], ident16[:])
                    nc.vector.tensor_copy(UqT[:, k, :], tq[:])


            YT = med.tile([128, NT, Q], bf16, tag="YT")
            for t in range(NT):
                nc.vector.tensor_scalar_mul(YT[:, t, :], UqT[:, t, :],
                                            w_cols[:, 2, t:t + 1])

            ST = ps_big.tile([128, D], f32, tag="pbig")
            for hf in range(2):
                for t in range(NT):
                    nc.tensor.matmul(ST[:, ts(hf, HHALF)], YT[:, t, :],
                                     UdT[:, t, ts(hf, HHALF)],
                                     start=(t == 0), stop=(t == NT - 1))

            sq_ps = ps_sm.tile([1, Q], f32, tag="psm")
            for t in range(NT):
                nc.tensor.matmul(sq_ps[:], wq16[:, t:t + 1], UqT[:, t, :],
                                 start=(t == 0), stop=(t == NT - 1))
            sq_row = vec.tile([1, Q], f32, tag="sqrow")
            nc.scalar.copy(sq_row[:], sq_ps[:])
            sqc_ps = ps_sm.tile([128, 1], f32, tag="psm")
            nc.tensor.transpose(sqc_ps[:], sq_row[:], ident1f[:])
            sqb = vec.tile([128, 1], f32, tag="sqb")
            nc.scalar.activation(sqb[:], sqc_ps[:], AF.Identity, bias=qbias[:])

            sdc_ps = ps_sm.tile([128, NT], f32, tag="psm")
            for hf in range(2):
                sd_ps = ps_sm.tile([1, HHALF], f32, tag="psm")
                for t in range(NT):
                    nc.tensor.matmul(sd_ps[:], wd16[:, t:t + 1],
                                     UdT[:, t, ts(hf, HHALF)],
                                     start=(t == 0), stop=(t == NT - 1))
                sd_row = vec.tile([1, HHALF], f32, tag="sdrow")
                nc.scalar.copy(sd_row[:], sd_ps[:])
                for j in range(4):
                    t = hf * 4 + j
                    nc.tensor.transpose(sdc_ps[:, t:t + 1],
                                        sd_row[0:1, ts(j, 128)], ident1f[:])
            exps = vec.tile([128, NT], f32, tag="exps")
            for t in range(NT):
                nc.scalar.activation(exps[:, t:t + 1], sdc_ps[:, t:t + 1],
                                     AF.Exp, bias=dbias[:, t:t + 1])
            exps16 = vec.tile([128, NT], bf16, tag="exps16")
            nc.vector.tensor_copy(exps16[:], exps[:])

            ET = med.tile([128, D], bf16, tag="ET")
            for hf in range(2):
                nc.scalar.activation(ET[:, ts(hf, HHALF)], ST[:, ts(hf, HHALF)],
                                     AF.Exp, bias=sqb[:])
            EN = med.tile([128, NT, Q], bf16, tag="EN")
            r_cols = vec.tile([128, NT], f32, tag="rcols")
            for ec in range(NT):
                en_ps = ps_sm.tile([128, Q], bf16, tag="psm")
                nc.tensor.transpose(en_ps[:], ET[:, ts(ec, 128)], ident16[:])
                nc.scalar.activation(EN[:, ec, :], en_ps[:], AF.Copy,
                                     accum_out=r_cols[:, ec:ec + 1])
            rinv = vec.tile([128, NT], f32, tag="rinv")
            nc.vector.reciprocal(rinv[:], r_cols[:])
            emit_ud_copy_out(range(4, NT) if bi % NB == 0 else range(NT))

            Ut = utp.tile([128, NT, H], bf16, tag="Ut")
            ut_src = Ud
            for t in range(NT):
                nc.vector.tensor_scalar_mul(Ut[:, t, :], ut_src[:, t, :],
                                            exps[:, t:t + 1])
            Wb = ps_big.tile([128, H], f32, tag="pbig")
            for hf in range(2):
                for et in range(NT):
                    nc.tensor.matmul(Wb[:, ts(hf, HHALF)], EN[:, et, :],
                                     Ut[:, et, ts(hf, HHALF)],
                                     start=(et == 0), stop=(et == NT - 1))
            c2_ps = ps_sm.tile([128, 1], f32, tag="psm")
            for et in range(NT):
                nc.tensor.matmul(c2_ps[:], EN[:, et, :], exps16[:, et:et + 1],
                                 start=(et == 0), stop=(et == NT - 1))
            c2inv = vec.tile([128, 1], f32, tag="c2inv")
            nc.vector.reciprocal(c2inv[:], c2_ps[:])
            W = med.tile([128, H], bf16, tag="W")
            for hf in range(2):
                nc.scalar.mul(W[:, ts(hf, HHALF)], Wb[:, ts(hf, HHALF)],
                              c2inv[:])

            for dc in range(NT):
                lhs = ET[:, ts(dc, 128)]
                rdc = rinv[:, dc:dc + 1]
                Ad = outp.tile([128, H], f32, tag="Ad")
                C3 = outp.tile([128, H], f32, tag="C3")
                C4 = outp.tile([128, H], f32, tag="C4")
                for hf in range(2):
                    a_ps = ps_mm.tile([128, HHALF], f32, tag="pmm")
                    nc.tensor.matmul(a_ps[:], lhs, Uq16[:, ts(hf, HHALF)],
                                     start=True, stop=True)
                    nc.scalar.mul(Ad[:, ts(hf, HHALF)], a_ps[:], rdc)
                    if "stt" not in skip:
                        nc.vector.scalar_tensor_tensor(
                            C3[:, ts(hf, HHALF)], a_ps[:], rdc,
                            Ud[:, dc, ts(hf, HHALF)], ALU.mult, ALU.mult)
                A4 = outp.tile([128, HHALF], f32, tag="A4")
                for hf in range(2):
                    r_ps = ps_mm.tile([128, HHALF], f32, tag="pmm")
                    nc.tensor.matmul(r_ps[:], lhs, W[:, ts(hf, HHALF)],
                                     start=True, stop=True)
                    if "stt" in skip:
                        continue
                    if c4 == "dve" or hf == 1:
                        nc.vector.scalar_tensor_tensor(
                            C4[:, ts(hf, HHALF)], r_ps[:], rdc,
                            Ud[:, dc, ts(hf, HHALF)], ALU.mult, ALU.mult)
                    else:
                        nc.scalar.mul(A4[:], r_ps[:], rdc)
                        nc.gpsimd.tensor_mul(
                            C4[:, ts(hf, HHALF)], A4[:],
                            Ud[:, dc, ts(hf, HHALF)])
                if "out_dma" not in skip:
                    rows = slice(dc * 128, (dc + 1) * 128)
                    for hf in range(2):
                        sl = slice(hf * HHALF, (hf + 1) * HHALF)
                        nc.sync.dma_start(
                            V_dram[b, rows, H + hf * HHALF:H + (hf + 1) * HHALF],
                            Ad[:, sl])
                        nc.sync.dma_start(
                            V_dram[b, rows, 2 * H + hf * HHALF:2 * H + (hf + 1) * HHALF],
                            C3[:, sl])
                        nc.sync.dma_start(
                            V_dram[b, rows, 3 * H + hf * HHALF:3 * H + (hf + 1) * HHALF],
                            C4[:, sl])

    nc.compile()
    return nc


def _get_nc():
    if 'nc' not in _CACHE:
        _CACHE['nc'] = build_nc()
    return _CACHE['nc']


def make_in_maps(inputs):
    U_d = np.asarray(inputs['U_d'], dtype=np.float32)
    U_q = np.asarray(inputs['U_q'], dtype=np.float32)
    wc_w = np.asarray(inputs['wc_w'], dtype=np.float32)
    q_mask = np.asarray(inputs['q_mask'], dtype=np.int32)
    d_mask = np.asarray(inputs['d_mask'], dtype=np.int32)
    w_cols = np.ascontiguousarray(
        wc_w.reshape(3, NT, 128).transpose(2, 0, 1))
    qbias = ((q_mask.astype(np.float32) - 1.0) * 30.0)[:, :, None]
    dbias = np.ascontiguousarray(
        ((d_mask.astype(np.float32) - 1.0) * 30.0)
        .reshape(B, NT, 128).transpose(0, 2, 1))
    in_maps = []
    for c in range(NCORES):
        s = slice(c * NB, (c + 1) * NB)
        in_maps.append({
            'U_d': U_d[s], 'U_q': U_q[s], 'wc_w': w_cols,
            'q_mask': qbias[s], 'd_mask': dbias[s],
        })
    return in_maps


def run(inputs, trace=False, **kw):
    from concourse.bass_utils import run_bass_kernel_spmd
    nc = _get_nc()
    res = run_bass_kernel_spmd(nc, make_in_maps(inputs), list(range(NCORES)),
                               trace=trace, **kw)
    out = np.concatenate([res.results[c]['V'] for c in range(NCORES)], axis=0)
    return out, res


def kernel(**inputs) -> np.ndarray:
    out, _ = run(inputs, trace=False)
    return out



# revision 19
# speedup vs baseline: 1.7517x; 1.7517x over previous
import sys

if '/opt/trn_rl_repo' not in sys.path:
    sys.path.insert(0, '/opt/trn_rl_repo')

import numpy as np

B, D, Q, H = 16, 1024, 128, 1024
NCORES = 8
NB = B // NCORES
NT = D // 128
HHALF = 512

_CACHE = {}

UDT_EVAC = "ADDADDDA"
UQT_EVAC = "D"
ENP_EVAC = "DDDDDDDD"
AD_ENG = {0: "AADAADAA", 1: "AADAADAA"}
C3_ENG = {0: "DDGDDGDD", 1: "DGGDGDGD"}
C4_ENG = {0: "DGMDDGMD",
          1: "DMMDDMMM"}


def build_nc():
    import concourse.bacc as bacc
    import concourse.tile as tile
    from concourse import mybir, masks
    import concourse.bass as bass
    from contextlib import ExitStack

    ts = bass.ts
    f32 = mybir.dt.float32
    bf16 = mybir.dt.bfloat16
    AF = mybir.ActivationFunctionType
    ALU = mybir.AluOpType

    nc = bacc.Bacc("TRN2", target_bir_lowering=False, debug=False)

    Ud_dram = nc.dram_tensor("U_d", [NB, D, H], bf16, kind="ExternalInput")
    Uq_dram = nc.dram_tensor("U_q", [NB, Q, H], bf16, kind="ExternalInput")
    w_dram = nc.dram_tensor("wc_w", [128, 3, NT], f32, kind="ExternalInput")
    mb_dram = nc.dram_tensor("d_mask", [NB, 128, NT + 1], f32,
                             kind="ExternalInput")
    V_dram = nc.dram_tensor("V", [NB, D, 4 * H], bf16, kind="ExternalOutput")

    with tile.TileContext(nc) as tc, ExitStack() as ctx:
        const = ctx.enter_context(tc.tile_pool(name="const", bufs=1))
        big = ctx.enter_context(tc.tile_pool(name="big", bufs=2))
        med = ctx.enter_context(tc.tile_pool(name="med", bufs=2))
        vec = ctx.enter_context(tc.tile_pool(name="vec", bufs=2))
        outp = ctx.enter_context(tc.tile_pool(name="outp", bufs=8))
        ps_pp = ctx.enter_context(tc.tile_pool(name="ps_pp", bufs=3, space="PSUM"))
        ps_sd = ctx.enter_context(tc.tile_pool(name="ps_sd", bufs=1, space="PSUM"))

        w_cols = const.tile([128, 3, NT], f32, tag="wcols")
        nc.gpsimd.dma_start(w_cols[:], w_dram[:])
        wd16 = const.tile([128, NT], bf16, tag="wd16")
        wq16 = const.tile([128, NT], bf16, tag="wq16")
        nc.vector.tensor_copy(wd16[:], w_cols[:, 0, :])
        nc.vector.tensor_copy(wq16[:], w_cols[:, 1, :])
        ident16 = const.tile([128, 128], bf16, tag="id16")
        masks.make_identity(nc, ident16[:])
        ones16 = const.tile([128, 1], bf16, tag="ones16")
        nc.vector.memset(ones16[:], 1.0)

        st = {}
        for b in range(NB):
            s = st[b] = {}
            Ud = s['Ud'] = big.tile([128, NT, H], bf16, tag="Ud", name=f"Ud{b}")
            Ud_src = Ud_dram[b].rearrange("(t p) h -> p t h", p=128)
            if b == 0:
                for q4 in range(4):
                    nc.sync.dma_start(Ud[:, 0, ts(q4, 256)],
                                      Ud_src[:, 0, ts(q4, 256)])
                for t in range(1, NT):
                    nc.sync.dma_start(Ud[:, t, :], Ud_src[:, t, :])
            else:
                nc.sync.dma_start(Ud[:], Ud_src[:])
            s['Uq16'] = med.tile([128, H], bf16, tag="Uq16", name=f"Uq16_{b}")
            nc.gpsimd.dma_start(s['Uq16'][:], Uq_dram[b])
            mk = s['mk'] = vec.tile([128, NT + 1], f32, tag="mk", name=f"mk{b}")
            nc.sync.dma_start(mk[:], mb_dram[b])

        def udsec(b, i8):
            rows = slice(i8 * 128, (i8 + 1) * 128)
            nc.sync.dma_start(V_dram[b, rows, 0:H], Ud_dram[b, rows, :])


        def stage_AB(b):
            s = st[b]
            Ud, Uq16, mk = s['Ud'], s['Uq16'], s['mk']
            qbias, dbias = mk[:, 0:1], mk[:, 1:NT + 1]

            UdT = s['UdT'] = big.tile([128, NT, D], bf16, tag="UdT", name=f"UdT{b}")
            UqT = med.tile([128, NT, Q], bf16, tag="UqT")
            for t in range(NT):
                tp = ps_pp.tile([128, NT * 128], bf16, tag="pp",
                                name=f"tp{b}_{t}", padded_shape=[128, 2048])
                for k in range(NT):
                    nc.tensor.transpose(tp[:, ts(k, 128)],
                                        Ud[:, t, ts(k, 128)], ident16[:])
                dst = UdT[:, :, ts(t, 128)]
                if UDT_EVAC[t] == 'A':
                    nc.scalar.copy(dst, tp[:])
                else:
                    nc.vector.tensor_copy(dst, tp[:])
            tq = ps_pp.tile([128, NT * 128], bf16, tag="pp", name=f"tq{b}",
                            padded_shape=[128, 2048])
            for k in range(NT):
                nc.tensor.transpose(tq[:, ts(k, 128)],
                                    Uq16[:, ts(k, 128)], ident16[:])
            if UQT_EVAC[0] == 'A':
                nc.scalar.copy(UqT[:], tq[:])
            else:
                nc.vector.tensor_copy(UqT[:], tq[:])

            YT = med.tile([128, NT, Q], bf16, tag="YT")
            for k in range(NT):
                nc.vector.tensor_scalar_mul(YT[:, k, :], UqT[:, k, :],
                                            w_cols[:, 2, k:k + 1])
            ST = ps_pp.tile([128, D], f32, tag="pp", name=f"ST{b}")
            for hf in range(2):
                for k in range(NT):
                    nc.tensor.matmul(ST[:, ts(hf, HHALF)], YT[:, k, :],
                                     UdT[:, k, ts(hf, HHALF)],
                                     start=(k == 0), stop=(k == NT - 1))

            smA = ps_sd.tile([128, 2 * NT + 1], f32, tag="smA")
            sdc_ps = smA[:, 0:NT]
            rc_ps = smA[:, NT:2 * NT]
            sqc_ps = smA[:, 2 * NT:2 * NT + 1]
            for dblk in range(NT):
                for k in range(NT):
                    nc.tensor.matmul(sdc_ps[:, dblk:dblk + 1],
                                     UdT[:, k, ts(dblk, 128)], wd16[:, k:k + 1],
                                     start=(k == 0), stop=(k == NT - 1))
            for k in range(NT):
                nc.tensor.matmul(sqc_ps[:], UqT[:, k, :], wq16[:, k:k + 1],
                                 start=(k == 0), stop=(k == NT - 1))
            sqb = vec.tile([128, 1], f32, tag="sqb")
            nc.scalar.activation(sqb[:], sqc_ps[:], AF.Identity, bias=qbias)
            sd_sum = vec.tile([128, NT], f32, tag="sdsum")
            nc.vector.tensor_add(sd_sum[:], sdc_ps[:], dbias)
            exps = s['exps'] = vec.tile([128, NT], f32, tag="exps", name=f"exps{b}")
            nc.scalar.activation(exps[:], sd_sum[:], AF.Exp)

            ET = s['ET'] = med.tile([128, D], bf16, tag="ET", name=f"ET{b}")
            for hf in range(2):
                nc.scalar.activation(ET[:, ts(hf, HHALF)], ST[:, ts(hf, HHALF)],
                                     AF.Exp, bias=sqb[:])
            for dc in range(NT):
                nc.tensor.matmul(rc_ps[:, dc:dc + 1], ET[:, ts(dc, 128)],
                                 ones16[:], start=True, stop=True)
            rinv = s['rinv'] = vec.tile([128, NT], f32, tag="rinv", name=f"rinv{b}")
            nc.vector.reciprocal(rinv[:], rc_ps[:])

        def stage_E1(b):
            s = st[b]
            Ud, Uq16, ET, rinv = s['Ud'], s['Uq16'], s['ET'], s['rinv']
            out2s = []
            for dc in range(NT):
                lhs = ET[:, ts(dc, 128)]
                rdc = rinv[:, dc:dc + 1]
                out2 = outp.tile([128, 2, H], bf16, tag="out2",
                                 name=f"out2_{b}_{dc}")
                out2s.append(out2)
                a_ps = ps_pp.tile([128, H], f32, tag="pp",
                                  name=f"aps{b}_{dc}")
                for hf in range(2):
                    nc.tensor.matmul(a_ps[:, ts(hf, HHALF)], lhs,
                                     Uq16[:, ts(hf, HHALF)],
                                     start=True, stop=True)
                if AD_ENG[b][dc] == 'A':
                    nc.scalar.mul(out2[:, 0, :], a_ps[:], rdc)
                else:
                    nc.vector.tensor_scalar_mul(out2[:, 0, :], a_ps[:], rdc)
            order = ([dc for dc in range(NT) if C3_ENG[b][dc] != 'G'] +
                     [dc for dc in range(NT) if C3_ENG[b][dc] == 'G'])
            for dc in order:
                out2 = out2s[dc]
                eng = nc.gpsimd if C3_ENG[b][dc] == 'G' else nc.vector
                eng.tensor_mul(out2[:, 1, :], out2[:, 0, :], Ud[:, dc, :])
            for dc in order:
                rows = slice(dc * 128, (dc + 1) * 128)
                nc.sync.dma_start(V_dram[b, rows, H:3 * H], out2s[dc][:])

        def stage_CD(b):
            s = st[b]
            Ud, ET, exps = s['Ud'], s['ET'], s['exps']
            EN = med.tile([128, NT, Q], bf16, tag="EN")
            te = ps_sd.tile([128, NT * 128], bf16, tag="te", name=f"te{b}")
            for ec in range(NT):
                nc.tensor.transpose(te[:, ts(ec, 128)],
                                    ET[:, ts(ec, 128)], ident16[:])
            for ec in range(NT):
                if ENP_EVAC[ec] == 'A':
                    nc.scalar.mul(EN[:, ec, :], te[:, ts(ec, 128)],
                                  exps[:, ec:ec + 1])
                else:
                    nc.vector.tensor_scalar_mul(EN[:, ec, :],
                                                te[:, ts(ec, 128)],
                                                exps[:, ec:ec + 1])
            Wb = ps_pp.tile([128, H], f32, tag="pp", name=f"Wb{b}")
            for hf in range(2):
                for et in range(NT):
                    nc.tensor.matmul(Wb[:, ts(hf, HHALF)], EN[:, et, :],
                                     Ud[:, et, ts(hf, HHALF)],
                                     start=(et == 0), stop=(et == NT - 1))
            smB = ps_pp.tile([128, 1], f32, tag="pp", name=f"c2_{b}",
                             padded_shape=[128, 1024])
            for et in range(NT):
                nc.tensor.matmul(smB[:], EN[:, et, :], ones16[:],
                                 start=(et == 0), stop=(et == NT - 1))
            c2inv = vec.tile([128, 1], f32, tag="c2inv")
            nc.vector.reciprocal(c2inv[:], smB[:])
            W = s['W'] = med.tile([128, H], bf16, tag="W", name=f"W{b}")
            nc.vector.tensor_scalar_mul(W[:], Wb[:], c2inv[:])

        def stage_E2(b):
            s = st[b]
            Ud, ET, rinv, W = s['Ud'], s['ET'], s['rinv'], s['W']
            out4s, a4s = [], {}
            for dc in range(NT):
                lhs = ET[:, ts(dc, 128)]
                rdc = rinv[:, dc:dc + 1]
                out4 = outp.tile([128, H], bf16, tag="out4",
                                 name=f"out4_{b}_{dc}")
                out4s.append(out4)
                r_ps = ps_pp.tile([128, H], f32, tag="pp",
                                  name=f"rps{b}_{dc}")
                for hf in range(2):
                    nc.tensor.matmul(r_ps[:, ts(hf, HHALF)], lhs,
                                     W[:, ts(hf, HHALF)],
                                     start=True, stop=True)
                if C4_ENG[b][dc] == 'D':
                    nc.vector.scalar_tensor_tensor(
                        out4[:], r_ps[:], rdc, Ud[:, dc, :],
                        ALU.mult, ALU.mult)
                else:
                    A4 = outp.tile([128, H], bf16, tag="A4",
                                   name=f"A4_{b}_{dc}")
                    nc.scalar.mul(A4[:], r_ps[:], rdc)
                    a4s[dc] = A4
            for dc in range(NT):
                if C4_ENG[b][dc] == 'D':
                    rows = slice(dc * 128, (dc + 1) * 128)
                    nc.sync.dma_start(V_dram[b, rows, 3 * H:4 * H],
                                      out4s[dc][:])
            if b == 1:
                udsec(1, 4)
                udsec(1, 5)
            order = ([dc for dc in range(NT) if C4_ENG[b][dc] == 'G'] +
                     [dc for dc in range(NT) if C4_ENG[b][dc] == 'M'])
            for dc in order:
                eng = nc.gpsimd if C4_ENG[b][dc] == 'G' else nc.vector
                eng.tensor_mul(out4s[dc][:], a4s[dc][:], Ud[:, dc, :])
            for n, dc in enumerate(order):
                rows = slice(dc * 128, (dc + 1) * 128)
                nc.sync.dma_start(V_dram[b, rows, 3 * H:4 * H], out4s[dc][:])
                if b == 1 and n < 2:
                    udsec(1, 6 + n)

        stage_AB(0)
        udsec(0, 0)
        udsec(0, 1)
        stage_AB(1)
        udsec(0, 2)
        udsec(0, 3)
        stage_E1(0)
        udsec(0, 4)
        stage_E1(1)
        udsec(0, 5)
        udsec(0, 6)
        stage_CD(0)
        udsec(0, 7)
        udsec(1, 0)
        stage_E2(0)
        udsec(1, 1)
        udsec(1, 2)
        udsec(1, 3)
        stage_CD(1)
        stage_E2(1)

    nc.compile()
    return nc


def _get_nc():
    if 'nc' not in _CACHE:
        _CACHE['nc'] = build_nc()
    return _CACHE['nc']


def make_in_maps(inputs):
    import ml_dtypes
    bf16 = ml_dtypes.bfloat16
    U_d = np.asarray(inputs['U_d'], dtype=np.float32).astype(bf16)
    U_q = np.asarray(inputs['U_q'], dtype=np.float32).astype(bf16)
    wc_w = np.asarray(inputs['wc_w'], dtype=np.float32)
    q_mask = np.asarray(inputs['q_mask'], dtype=np.int32)
    d_mask = np.asarray(inputs['d_mask'], dtype=np.int32)
    w_cols = np.ascontiguousarray(
        wc_w.reshape(3, NT, 128).transpose(2, 0, 1))
    qbias = ((q_mask.astype(np.float32) - 1.0) * 30.0)[:, :, None]
    dbias = ((d_mask.astype(np.float32) - 1.0) * 30.0) \
        .reshape(B, NT, 128).transpose(0, 2, 1)
    mbias = np.ascontiguousarray(
        np.concatenate([qbias, dbias], axis=2))
    in_maps = []
    for c in range(NCORES):
        s = slice(c * NB, (c + 1) * NB)
        in_maps.append({
            'U_d': np.ascontiguousarray(U_d[s]),
            'U_q': np.ascontiguousarray(U_q[s]),
            'wc_w': w_cols,
            'd_mask': mbias[s],
        })
    return in_maps


def run(inputs, trace=False, **kw):
    from concourse.bass_utils import run_bass_kernel_spmd
    nc = _get_nc()
    res = run_bass_kernel_spmd(nc, make_in_maps(inputs), list(range(NCORES)),
                               trace=trace, **kw)
    out = np.concatenate(
        [np.asarray(res.results[c]['V']).astype(np.float32)
         for c in range(NCORES)], axis=0)
    return out, res


def kernel(**inputs) -> np.ndarray:
    out, _ = run(inputs, trace=False)
    return out


# revision 24
# speedup vs baseline: 1.7783x; 1.0152x over previous
import sys

if '/opt/trn_rl_repo' not in sys.path:
    sys.path.insert(0, '/opt/trn_rl_repo')

import numpy as np

B, D, Q, H = 16, 1024, 128, 1024
NCORES = 8
NB = B // NCORES
NT = D // 128
HHALF = 512

_CACHE = {}

UDT_EVAC = "ADDADDDA"
UQT_EVAC = "D"
ENP_EVAC = "DDDDDDDD"
AD_ENG = {0: "AADAADAA", 1: "AADAADAA"}
C3_ENG = {0: "DDGDDGDD", 1: "DGGDGDGD"}
C4_ENG = {0: "DGMDDGMD",
          1: "DMMDDMMM"}


def build_nc():
    import concourse.bacc as bacc
    import concourse.tile as tile
    from concourse import mybir, masks
    import concourse.bass as bass
    from contextlib import ExitStack

    ts = bass.ts
    f32 = mybir.dt.float32
    bf16 = mybir.dt.bfloat16
    AF = mybir.ActivationFunctionType
    ALU = mybir.AluOpType

    nc = bacc.Bacc("TRN2", target_bir_lowering=False, debug=False)

    Ud_dram = nc.dram_tensor("U_d", [NB, D, H], bf16, kind="ExternalInput")
    Uq_dram = nc.dram_tensor("U_q", [NB, Q, H], bf16, kind="ExternalInput")
    w_dram = nc.dram_tensor("wc_w", [128, 3, NT], f32, kind="ExternalInput")
    mb_dram = nc.dram_tensor("d_mask", [NB, 128, NT + 1], f32,
                             kind="ExternalInput")
    V_dram = nc.dram_tensor("V", [NB, D, 4 * H], bf16, kind="ExternalOutput")

    with tile.TileContext(nc) as tc, ExitStack() as ctx:
        const = ctx.enter_context(tc.tile_pool(name="const", bufs=1))
        big = ctx.enter_context(tc.tile_pool(name="big", bufs=2))
        med = ctx.enter_context(tc.tile_pool(name="med", bufs=2))
        vec = ctx.enter_context(tc.tile_pool(name="vec", bufs=2))
        outp = ctx.enter_context(tc.tile_pool(name="outp", bufs=8))
        ps_pp = ctx.enter_context(tc.tile_pool(name="ps_pp", bufs=3, space="PSUM"))
        ps_sd = ctx.enter_context(tc.tile_pool(name="ps_sd", bufs=1, space="PSUM"))

        w_cols = const.tile([128, 3, NT], f32, tag="wcols")
        nc.gpsimd.dma_start(w_cols[:], w_dram[:])
        wd16 = const.tile([128, NT], bf16, tag="wd16")
        wq16 = const.tile([128, NT], bf16, tag="wq16")
        nc.vector.tensor_copy(wd16[:], w_cols[:, 0, :])
        nc.vector.tensor_copy(wq16[:], w_cols[:, 1, :])
        ident16 = const.tile([128, 128], bf16, tag="id16")
        masks.make_identity(nc, ident16[:])
        ones16 = const.tile([128, 1], bf16, tag="ones16")
        nc.vector.memset(ones16[:], 1.0)

        st = {}
        for b in range(NB):
            s = st[b] = {}
            Ud = s['Ud'] = big.tile([128, NT, H], bf16, tag="Ud", name=f"Ud{b}")
            Ud_src = Ud_dram[b].rearrange("(t p) h -> p t h", p=128)
            if b == 0:
                for q4 in range(4):
                    nc.sync.dma_start(Ud[:, 0, ts(q4, 256)],
                                      Ud_src[:, 0, ts(q4, 256)])
                for t in range(1, NT):
                    nc.sync.dma_start(Ud[:, t, :], Ud_src[:, t, :])
            else:
                nc.sync.dma_start(Ud[:], Ud_src[:])
            s['Uq16'] = med.tile([128, H], bf16, tag="Uq16", name=f"Uq16_{b}")
            nc.gpsimd.dma_start(s['Uq16'][:], Uq_dram[b])
            mk = s['mk'] = vec.tile([128, NT + 1], f32, tag="mk", name=f"mk{b}")
            nc.sync.dma_start(mk[:], mb_dram[b])

        def udsec(b, i8):
            rows = slice(i8 * 128, (i8 + 1) * 128)
            nc.sync.dma_start(V_dram[b, rows, 0:H], Ud_dram[b, rows, :])


        def ab_setup(b):
            s = st[b]
            s['UdT'] = big.tile([128, NT, D], bf16, tag="UdT", name=f"UdT{b}")
            s['UqT'] = med.tile([128, NT, Q], bf16, tag="UqT", name=f"UqT{b}")
            s['YT'] = med.tile([128, NT, Q], bf16, tag="YT", name=f"YT{b}")
            s['ET'] = med.tile([128, D], bf16, tag="ET", name=f"ET{b}")
            s['rinv'] = vec.tile([128, NT], f32, tag="rinv", name=f"rinv{b}")
            s['smA'] = ps_sd.tile([128, 2 * NT + 1], f32, tag="smA",
                                  name=f"smA{b}")
            s['sqb'] = vec.tile([128, 1], f32, tag="sqb", name=f"sqb{b}")
            s['ST'] = [None, None]

        def ab_uq(b):
            s = st[b]
            Uq16, UqT, YT, mk = s['Uq16'], s['UqT'], s['YT'], s['mk']
            tq = ps_pp.tile([128, NT * 128], bf16, tag="pp", name=f"tq{b}",
                            padded_shape=[128, 2048])
            for k in range(NT):
                nc.tensor.transpose(tq[:, ts(k, 128)],
                                    Uq16[:, ts(k, 128)], ident16[:])
            if UQT_EVAC[0] == 'A':
                nc.scalar.copy(UqT[:], tq[:])
            else:
                nc.vector.tensor_copy(UqT[:], tq[:])
            for k in range(NT):
                nc.vector.tensor_scalar_mul(YT[:, k, :], UqT[:, k, :],
                                            w_cols[:, 2, k:k + 1])
            sqc_ps = s['smA'][:, 2 * NT:2 * NT + 1]
            for k in range(NT):
                nc.tensor.matmul(sqc_ps[:], UqT[:, k, :], wq16[:, k:k + 1],
                                 start=(k == 0), stop=(k == NT - 1))
            nc.scalar.activation(s['sqb'][:], sqc_ps[:], AF.Identity,
                                 bias=mk[:, 0:1])

        def ab_half(b, hf):
            s = st[b]
            Ud, UdT, YT, ET = s['Ud'], s['UdT'], s['YT'], s['ET']
            for t in range(4 * hf, 4 * hf + 4):
                tp = ps_pp.tile([128, NT * 128], bf16, tag="pp",
                                name=f"tp{b}_{t}", padded_shape=[128, 2048])
                for k in range(NT):
                    nc.tensor.transpose(tp[:, ts(k, 128)],
                                        Ud[:, t, ts(k, 128)], ident16[:])
                dst = UdT[:, :, ts(t, 128)]
                if UDT_EVAC[t] == 'A':
                    nc.scalar.copy(dst, tp[:])
                else:
                    nc.vector.tensor_copy(dst, tp[:])
            STh = ps_pp.tile([128, HHALF], f32, tag="pp", name=f"ST{b}_{hf}",
                             padded_shape=[128, 1024])
            s['ST'][hf] = STh
            for k in range(NT):
                nc.tensor.matmul(STh[:], YT[:, k, :],
                                 UdT[:, k, ts(hf, HHALF)],
                                 start=(k == 0), stop=(k == NT - 1))
            nc.scalar.activation(ET[:, ts(hf, HHALF)], STh[:], AF.Exp,
                                 bias=s['sqb'][:])
            rc_ps = s['smA'][:, NT:2 * NT]
            for dc in range(4 * hf, 4 * hf + 4):
                nc.tensor.matmul(rc_ps[:, dc:dc + 1], ET[:, ts(dc, 128)],
                                 ones16[:], start=True, stop=True)
            nc.vector.reciprocal(s['rinv'][:, 4 * hf:4 * hf + 4],
                                 rc_ps[:, 4 * hf:4 * hf + 4])

        def ab_sd(b):
            s = st[b]
            UdT, mk = s['UdT'], s['mk']
            sdc_ps = s['smA'][:, 0:NT]
            for dblk in range(NT):
                for k in range(NT):
                    nc.tensor.matmul(sdc_ps[:, dblk:dblk + 1],
                                     UdT[:, k, ts(dblk, 128)], wd16[:, k:k + 1],
                                     start=(k == 0), stop=(k == NT - 1))
            sd_sum = vec.tile([128, NT], f32, tag="sdsum", name=f"sds{b}")
            nc.vector.tensor_add(sd_sum[:], sdc_ps[:], mk[:, 1:NT + 1])
            exps = s['exps'] = vec.tile([128, NT], f32, tag="exps",
                                        name=f"exps{b}")
            nc.scalar.activation(exps[:], sd_sum[:], AF.Exp)

        def stage_AB(b):
            ab_setup(b)
            s = st[b]
            Ud, UdT, UqT, YT = s['Ud'], s['UdT'], s['UqT'], s['YT']
            Uq16, mk, ET = s['Uq16'], s['mk'], s['ET']
            for t in range(NT):
                tp = ps_pp.tile([128, NT * 128], bf16, tag="pp",
                                name=f"tp{b}_{t}", padded_shape=[128, 2048])
                for k in range(NT):
                    nc.tensor.transpose(tp[:, ts(k, 128)],
                                        Ud[:, t, ts(k, 128)], ident16[:])
                dst = UdT[:, :, ts(t, 128)]
                if UDT_EVAC[t] == 'A':
                    nc.scalar.copy(dst, tp[:])
                else:
                    nc.vector.tensor_copy(dst, tp[:])
            tq = ps_pp.tile([128, NT * 128], bf16, tag="pp", name=f"tq{b}",
                            padded_shape=[128, 2048])
            for k in range(NT):
                nc.tensor.transpose(tq[:, ts(k, 128)],
                                    Uq16[:, ts(k, 128)], ident16[:])
            if UQT_EVAC[0] == 'A':
                nc.scalar.copy(UqT[:], tq[:])
            else:
                nc.vector.tensor_copy(UqT[:], tq[:])
            for k in range(NT):
                nc.vector.tensor_scalar_mul(YT[:, k, :], UqT[:, k, :],
                                            w_cols[:, 2, k:k + 1])
            ST = ps_pp.tile([128, D], f32, tag="pp", name=f"ST{b}")
            for hf in range(2):
                for k in range(NT):
                    nc.tensor.matmul(ST[:, ts(hf, HHALF)], YT[:, k, :],
                                     UdT[:, k, ts(hf, HHALF)],
                                     start=(k == 0), stop=(k == NT - 1))
            smA = s['smA']
            sdc_ps, rc_ps = smA[:, 0:NT], smA[:, NT:2 * NT]
            sqc_ps = smA[:, 2 * NT:2 * NT + 1]
            for dblk in range(NT):
                for k in range(NT):
                    nc.tensor.matmul(sdc_ps[:, dblk:dblk + 1],
                                     UdT[:, k, ts(dblk, 128)], wd16[:, k:k + 1],
                                     start=(k == 0), stop=(k == NT - 1))
            for k in range(NT):
                nc.tensor.matmul(sqc_ps[:], UqT[:, k, :], wq16[:, k:k + 1],
                                 start=(k == 0), stop=(k == NT - 1))
            nc.scalar.activation(s['sqb'][:], sqc_ps[:], AF.Identity,
                                 bias=mk[:, 0:1])
            sd_sum = vec.tile([128, NT], f32, tag="sdsum", name=f"sds{b}")
            nc.vector.tensor_add(sd_sum[:], sdc_ps[:], mk[:, 1:NT + 1])
            exps = s['exps'] = vec.tile([128, NT], f32, tag="exps",
                                        name=f"exps{b}")
            nc.scalar.activation(exps[:], sd_sum[:], AF.Exp)
            for hf in range(2):
                nc.scalar.activation(ET[:, ts(hf, HHALF)], ST[:, ts(hf, HHALF)],
                                     AF.Exp, bias=s['sqb'][:])
            for dc in range(NT):
                nc.tensor.matmul(rc_ps[:, dc:dc + 1], ET[:, ts(dc, 128)],
                                 ones16[:], start=True, stop=True)
            nc.vector.reciprocal(s['rinv'][:], rc_ps[:])

        def stage_E1(b, fill=(), dcs=tuple(range(NT))):
            s = st[b]
            Ud, Uq16, ET, rinv = s['Ud'], s['Uq16'], s['ET'], s['rinv']
            out2s = {}
            for dc in dcs:
                lhs = ET[:, ts(dc, 128)]
                rdc = rinv[:, dc:dc + 1]
                out2 = outp.tile([128, 2, H], bf16, tag="out2",
                                 name=f"out2_{b}_{dc}")
                out2s[dc] = out2
                a_ps = ps_pp.tile([128, H], f32, tag="pp",
                                  name=f"aps{b}_{dc}")
                for hf in range(2):
                    nc.tensor.matmul(a_ps[:, ts(hf, HHALF)], lhs,
                                     Uq16[:, ts(hf, HHALF)],
                                     start=True, stop=True)
                if AD_ENG[b][dc] == 'A':
                    nc.scalar.mul(out2[:, 0, :], a_ps[:], rdc)
                else:
                    nc.vector.tensor_scalar_mul(out2[:, 0, :], a_ps[:], rdc)
            order = ([dc for dc in dcs if C3_ENG[b][dc] != 'G'] +
                     [dc for dc in dcs if C3_ENG[b][dc] == 'G'])
            for dc in order:
                out2 = out2s[dc]
                eng = nc.gpsimd if C3_ENG[b][dc] == 'G' else nc.vector
                eng.tensor_mul(out2[:, 1, :], out2[:, 0, :], Ud[:, dc, :])
            for n, dc in enumerate(order):
                rows = slice(dc * 128, (dc + 1) * 128)
                nc.sync.dma_start(V_dram[b, rows, H:3 * H], out2s[dc][:])
                for fb, fi in dict(fill).get(n, ()):
                    udsec(fb, fi)

        def stage_CD(b):
            s = st[b]
            Ud, ET, exps = s['Ud'], s['ET'], s['exps']
            EN = med.tile([128, NT, Q], bf16, tag="EN")
            te = ps_sd.tile([128, NT * 128], bf16, tag="te", name=f"te{b}")
            for ec in range(NT):
                nc.tensor.transpose(te[:, ts(ec, 128)],
                                    ET[:, ts(ec, 128)], ident16[:])
            for ec in range(NT):
                if ENP_EVAC[ec] == 'A':
                    nc.scalar.mul(EN[:, ec, :], te[:, ts(ec, 128)],
                                  exps[:, ec:ec + 1])
                else:
                    nc.vector.tensor_scalar_mul(EN[:, ec, :],
                                                te[:, ts(ec, 128)],
                                                exps[:, ec:ec + 1])
            Wb = ps_pp.tile([128, H], f32, tag="pp", name=f"Wb{b}")
            for hf in range(2):
                for et in range(NT):
                    nc.tensor.matmul(Wb[:, ts(hf, HHALF)], EN[:, et, :],
                                     Ud[:, et, ts(hf, HHALF)],
                                     start=(et == 0), stop=(et == NT - 1))
            smB = ps_pp.tile([128, 1], f32, tag="pp", name=f"c2_{b}",
                             padded_shape=[128, 1024])
            for et in range(NT):
                nc.tensor.matmul(smB[:], EN[:, et, :], ones16[:],
                                 start=(et == 0), stop=(et == NT - 1))
            c2inv = vec.tile([128, 1], f32, tag="c2inv")
            nc.vector.reciprocal(c2inv[:], smB[:])
            W = s['W'] = med.tile([128, H], bf16, tag="W", name=f"W{b}")
            nc.vector.tensor_scalar_mul(W[:], Wb[:], c2inv[:])

        def stage_E2(b, fill=()):
            s = st[b]
            Ud, ET, rinv, W = s['Ud'], s['ET'], s['rinv'], s['W']
            out4s, a4s = [], {}
            for dc in range(NT):
                lhs = ET[:, ts(dc, 128)]
                rdc = rinv[:, dc:dc + 1]
                out4 = outp.tile([128, H], bf16, tag="out4",
                                 name=f"out4_{b}_{dc}")
                out4s.append(out4)
                r_ps = ps_pp.tile([128, H], f32, tag="pp",
                                  name=f"rps{b}_{dc}")
                for hf in range(2):
                    nc.tensor.matmul(r_ps[:, ts(hf, HHALF)], lhs,
                                     W[:, ts(hf, HHALF)],
                                     start=True, stop=True)
                if C4_ENG[b][dc] == 'D':
                    nc.vector.scalar_tensor_tensor(
                        out4[:], r_ps[:], rdc, Ud[:, dc, :],
                        ALU.mult, ALU.mult)
                else:
                    A4 = outp.tile([128, H], bf16, tag="A4",
                                   name=f"A4_{b}_{dc}")
                    nc.scalar.mul(A4[:], r_ps[:], rdc)
                    a4s[dc] = A4
            nD = 0
            for dc in range(NT):
                if C4_ENG[b][dc] == 'D':
                    rows = slice(dc * 128, (dc + 1) * 128)
                    nc.sync.dma_start(V_dram[b, rows, 3 * H:4 * H],
                                      out4s[dc][:])
                    nD += 1
                    for fb, fi in dict(fill).get(nD - 1, ()):
                        udsec(fb, fi)
            order = ([dc for dc in range(NT) if C4_ENG[b][dc] == 'G'] +
                     [dc for dc in range(NT) if C4_ENG[b][dc] == 'M'])
            for dc in order:
                eng = nc.gpsimd if C4_ENG[b][dc] == 'G' else nc.vector
                eng.tensor_mul(out4s[dc][:], a4s[dc][:], Ud[:, dc, :])
            for n, dc in enumerate(order):
                rows = slice(dc * 128, (dc + 1) * 128)
                nc.sync.dma_start(V_dram[b, rows, 3 * H:4 * H], out4s[dc][:])
                for fb, fi in dict(fill).get(('m', n), ()):
                    udsec(fb, fi)

        stage_AB(0)
        udsec(0, 0)
        stage_E1(0, fill={1: [(0, 1)], 3: [(0, 2)], 5: [(0, 3)]})
        stage_AB(1)
        udsec(0, 4)
        stage_E1(1, fill={1: [(0, 5)], 3: [(0, 6)]})
        udsec(0, 7)
        udsec(1, 0)
        stage_CD(0)
        udsec(1, 1)
        udsec(1, 2)
        stage_E2(0, fill={2: [(1, 3)], ('m', 0): [(1, 4)]})
        udsec(1, 5)
        stage_CD(1)
        stage_E2(1, fill={1: [(1, 6)], ('m', 0): [(1, 7)]})

    nc.compile()
    return nc


def _get_nc():
    if 'nc' not in _CACHE:
        _CACHE['nc'] = build_nc()
    return _CACHE['nc']


def make_in_maps(inputs):
    import ml_dtypes
    bf16 = ml_dtypes.bfloat16
    U_d = np.asarray(inputs['U_d'], dtype=np.float32).astype(bf16)
    U_q = np.asarray(inputs['U_q'], dtype=np.float32).astype(bf16)
    wc_w = np.asarray(inputs['wc_w'], dtype=np.float32)
    q_mask = np.asarray(inputs['q_mask'], dtype=np.int32)
    d_mask = np.asarray(inputs['d_mask'], dtype=np.int32)
    w_cols = np.ascontiguousarray(
        wc_w.reshape(3, NT, 128).transpose(2, 0, 1))
    qbias = ((q_mask.astype(np.float32) - 1.0) * 30.0)[:, :, None]
    dbias = ((d_mask.astype(np.float32) - 1.0) * 30.0) \
        .reshape(B, NT, 128).transpose(0, 2, 1)
    mbias = np.ascontiguousarray(
        np.concatenate([qbias, dbias], axis=2))
    in_maps = []
    for c in range(NCORES):
        s = slice(c * NB, (c + 1) * NB)
        in_maps.append({
            'U_d': np.ascontiguousarray(U_d[s]),
            'U_q': np.ascontiguousarray(U_q[s]),
            'wc_w': w_cols,
            'd_mask': mbias[s],
        })
    return in_maps


def run(inputs, trace=False, **kw):
    from concourse.bass_utils import run_bass_kernel_spmd
    nc = _get_nc()
    res = run_bass_kernel_spmd(nc, make_in_maps(inputs), list(range(NCORES)),
                               trace=trace, **kw)
    out = np.concatenate(
        [np.asarray(res.results[c]['V']).astype(np.float32)
         for c in range(NCORES)], axis=0)
    return out, res


def kernel(**inputs) -> np.ndarray:
    out, _ = run(inputs, trace=False)
    return out


# revision 32
# speedup vs baseline: 1.8046x; 1.0148x over previous
import sys

if '/opt/trn_rl_repo' not in sys.path:
    sys.path.insert(0, '/opt/trn_rl_repo')

import numpy as np

B, D, Q, H = 16, 1024, 128, 1024
NCORES = 8
NB = B // NCORES
NT = D // 128
HHALF = 512

_CACHE = {}

UDT_EVAC = "ADDADDDA"
UQT_EVAC = "D"
ENP_EVAC = "DDDDDDDD"
AD_ENG = {0: "AADAADAA", 1: "AADAADAA"}
C3_ENG = {0: "DDGDDGDD", 1: "DGGDGDGD"}
C4_ENG = {0: "DGMDDGMD",
          1: "DMGDDGMD"}


def build_nc():
    import concourse.bacc as bacc
    import concourse.tile as tile
    from concourse import mybir, masks
    import concourse.bass as bass
    from contextlib import ExitStack

    ts = bass.ts
    f32 = mybir.dt.float32
    bf16 = mybir.dt.bfloat16
    AF = mybir.ActivationFunctionType
    ALU = mybir.AluOpType

    nc = bacc.Bacc("TRN2", target_bir_lowering=False, debug=False)

    Ud_dram = nc.dram_tensor("U_d", [NB, D, H], bf16, kind="ExternalInput")
    Uq_dram = nc.dram_tensor("U_q", [NB, Q, H], bf16, kind="ExternalInput")
    w_dram = nc.dram_tensor("wc_w", [128, 3, NT], f32, kind="ExternalInput")
    mb_dram = nc.dram_tensor("d_mask", [NB, 128, NT + 1], f32,
                             kind="ExternalInput")
    V_dram = nc.dram_tensor("V", [NB, D, 4 * H], bf16, kind="ExternalOutput")

    with tile.TileContext(nc) as tc, ExitStack() as ctx:
        const = ctx.enter_context(tc.tile_pool(name="const", bufs=1))
        big = ctx.enter_context(tc.tile_pool(name="big", bufs=2))
        med = ctx.enter_context(tc.tile_pool(name="med", bufs=2))
        vec = ctx.enter_context(tc.tile_pool(name="vec", bufs=2))
        outp = ctx.enter_context(tc.tile_pool(name="outp", bufs=8))
        ps_pp = ctx.enter_context(tc.tile_pool(name="ps_pp", bufs=3, space="PSUM"))
        ps_sd = ctx.enter_context(tc.tile_pool(name="ps_sd", bufs=1, space="PSUM"))

        w_cols = const.tile([128, 3, NT], f32, tag="wcols")
        nc.gpsimd.dma_start(w_cols[:], w_dram[:])
        wd16 = const.tile([128, NT], bf16, tag="wd16")
        wq16 = const.tile([128, NT], bf16, tag="wq16")
        nc.vector.tensor_copy(wd16[:], w_cols[:, 0, :])
        nc.vector.tensor_copy(wq16[:], w_cols[:, 1, :])
        ident16 = const.tile([128, 128], bf16, tag="id16")
        masks.make_identity(nc, ident16[:])
        ones16 = const.tile([128, 1], bf16, tag="ones16")
        nc.vector.memset(ones16[:], 1.0)

        st = {}
        for b in range(NB):
            s = st[b] = {}
            Ud = s['Ud'] = big.tile([128, NT, H], bf16, tag="Ud", name=f"Ud{b}")
            Ud_src = Ud_dram[b].rearrange("(t p) h -> p t h", p=128)
            if b == 0:
                for q2 in range(2):
                    nc.sync.dma_start(Ud[:, 0, ts(q2, 512)],
                                      Ud_src[:, 0, ts(q2, 512)])
                for t in range(1, NT):
                    nc.sync.dma_start(Ud[:, t, :], Ud_src[:, t, :])
            else:
                nc.sync.dma_start(Ud[:, 0:4, :], Ud_src[:, 0:4, :])
                st[b]['dma_late'] = (Ud[:, 4:NT, :], Ud_src[:, 4:NT, :])
            s['Uq16'] = med.tile([128, H], bf16, tag="Uq16", name=f"Uq16_{b}")
            nc.gpsimd.dma_start(s['Uq16'][:], Uq_dram[b])
            mk = s['mk'] = vec.tile([128, NT + 1], f32, tag="mk", name=f"mk{b}")
            nc.sync.dma_start(mk[:], mb_dram[b])

        def udsec(b, i8):
            rows = slice(i8 * 128, (i8 + 1) * 128)
            nc.sync.dma_start(V_dram[b, rows, 0:H], Ud_dram[b, rows, :])


        def ab_setup(b):
            s = st[b]
            s['UdT'] = big.tile([128, NT, D], bf16, tag="UdT", name=f"UdT{b}")
            s['UqT'] = med.tile([128, NT, Q], bf16, tag="UqT", name=f"UqT{b}")
            s['YT'] = med.tile([128, NT, Q], bf16, tag="YT", name=f"YT{b}")
            s['ET'] = med.tile([128, D], bf16, tag="ET", name=f"ET{b}")
            s['rinv'] = vec.tile([128, NT], f32, tag="rinv", name=f"rinv{b}")
            s['smA'] = ps_sd.tile([128, 2 * NT + 1], f32, tag="smA",
                                  name=f"smA{b}")
            s['sqb'] = vec.tile([128, 1], f32, tag="sqb", name=f"sqb{b}")
            s['ST'] = [None, None]

        def ab_uq(b):
            s = st[b]
            Uq16, UqT, YT, mk = s['Uq16'], s['UqT'], s['YT'], s['mk']
            tq = ps_pp.tile([128, NT * 128], bf16, tag="pp", name=f"tq{b}",
                            padded_shape=[128, 2048])
            for k in range(NT):
                nc.tensor.transpose(tq[:, ts(k, 128)],
                                    Uq16[:, ts(k, 128)], ident16[:])
            if UQT_EVAC[0] == 'A':
                nc.scalar.copy(UqT[:], tq[:])
            else:
                nc.vector.tensor_copy(UqT[:], tq[:])
            for k in range(NT):
                nc.vector.tensor_scalar_mul(YT[:, k, :], UqT[:, k, :],
                                            w_cols[:, 2, k:k + 1])
            sqc_ps = s['smA'][:, 2 * NT:2 * NT + 1]
            for k in range(NT):
                nc.tensor.matmul(sqc_ps[:], UqT[:, k, :], wq16[:, k:k + 1],
                                 start=(k == 0), stop=(k == NT - 1))
            nc.scalar.activation(s['sqb'][:], sqc_ps[:], AF.Identity,
                                 bias=mk[:, 0:1])

        def ab_half(b, hf):
            s = st[b]
            Ud, UdT, YT, ET = s['Ud'], s['UdT'], s['YT'], s['ET']
            for t in range(4 * hf, 4 * hf + 4):
                tp = ps_pp.tile([128, NT * 128], bf16, tag="pp",
                                name=f"tp{b}_{t}", padded_shape=[128, 2048])
                for k in range(NT):
                    nc.tensor.transpose(tp[:, ts(k, 128)],
                                        Ud[:, t, ts(k, 128)], ident16[:])
                dst = UdT[:, :, ts(t, 128)]
                if UDT_EVAC[t] == 'A':
                    nc.scalar.copy(dst, tp[:])
                else:
                    nc.vector.tensor_copy(dst, tp[:])
            STh = ps_pp.tile([128, HHALF], f32, tag="pp", name=f"ST{b}_{hf}",
                             padded_shape=[128, 1024])
            s['ST'][hf] = STh
            for k in range(NT):
                nc.tensor.matmul(STh[:], YT[:, k, :],
                                 UdT[:, k, ts(hf, HHALF)],
                                 start=(k == 0), stop=(k == NT - 1))
            nc.scalar.activation(ET[:, ts(hf, HHALF)], STh[:], AF.Exp,
                                 bias=s['sqb'][:])
            rc_ps = s['smA'][:, NT:2 * NT]
            for dc in range(4 * hf, 4 * hf + 4):
                nc.tensor.matmul(rc_ps[:, dc:dc + 1], ET[:, ts(dc, 128)],
                                 ones16[:], start=True, stop=True)
            nc.vector.reciprocal(s['rinv'][:, 4 * hf:4 * hf + 4],
                                 rc_ps[:, 4 * hf:4 * hf + 4])

        def ab_sd(b):
            s = st[b]
            UdT, mk = s['UdT'], s['mk']
            sdc_ps = s['smA'][:, 0:NT]
            for dblk in range(NT):
                for k in range(NT):
                    nc.tensor.matmul(sdc_ps[:, dblk:dblk + 1],
                                     UdT[:, k, ts(dblk, 128)], wd16[:, k:k + 1],
                                     start=(k == 0), stop=(k == NT - 1))
            sd_sum = vec.tile([128, NT], f32, tag="sdsum", name=f"sds{b}")
            nc.vector.tensor_add(sd_sum[:], sdc_ps[:], mk[:, 1:NT + 1])
            exps = s['exps'] = vec.tile([128, NT], f32, tag="exps",
                                        name=f"exps{b}")
            nc.scalar.activation(exps[:], sd_sum[:], AF.Exp)

        def stage_AB(b):
            ab_setup(b)
            s = st[b]
            Ud, UdT, UqT, YT = s['Ud'], s['UdT'], s['UqT'], s['YT']
            Uq16, mk, ET = s['Uq16'], s['mk'], s['ET']
            for t in range(NT):
                tp = ps_pp.tile([128, NT * 128], bf16, tag="pp",
                                name=f"tp{b}_{t}", padded_shape=[128, 2048])
                for k in range(NT):
                    nc.tensor.transpose(tp[:, ts(k, 128)],
                                        Ud[:, t, ts(k, 128)], ident16[:])
                dst = UdT[:, :, ts(t, 128)]
                if UDT_EVAC[t] == 'A':
                    nc.scalar.copy(dst, tp[:])
                else:
                    nc.vector.tensor_copy(dst, tp[:])
            tq = ps_pp.tile([128, NT * 128], bf16, tag="pp", name=f"tq{b}",
                            padded_shape=[128, 2048])
            for k in range(NT):
                nc.tensor.transpose(tq[:, ts(k, 128)],
                                    Uq16[:, ts(k, 128)], ident16[:])
            if UQT_EVAC[0] == 'A':
                nc.scalar.copy(UqT[:], tq[:])
            else:
                nc.vector.tensor_copy(UqT[:], tq[:])
            for k in range(NT):
                nc.vector.tensor_scalar_mul(YT[:, k, :], UqT[:, k, :],
                                            w_cols[:, 2, k:k + 1])
            ST = ps_pp.tile([128, D], f32, tag="pp", name=f"ST{b}")
            for hf in range(2):
                for k in range(NT):
                    nc.tensor.matmul(ST[:, ts(hf, HHALF)], YT[:, k, :],
                                     UdT[:, k, ts(hf, HHALF)],
                                     start=(k == 0), stop=(k == NT - 1))
            smA = s['smA']
            sdc_ps, rc_ps = smA[:, 0:NT], smA[:, NT:2 * NT]
            sqc_ps = smA[:, 2 * NT:2 * NT + 1]
            for dblk in range(NT):
                for k in range(NT):
                    nc.tensor.matmul(sdc_ps[:, dblk:dblk + 1],
                                     UdT[:, k, ts(dblk, 128)], wd16[:, k:k + 1],
                                     start=(k == 0), stop=(k == NT - 1))
            for k in range(NT):
                nc.tensor.matmul(sqc_ps[:], UqT[:, k, :], wq16[:, k:k + 1],
                                 start=(k == 0), stop=(k == NT - 1))
            nc.scalar.activation(s['sqb'][:], sqc_ps[:], AF.Identity,
                                 bias=mk[:, 0:1])
            sd_sum = vec.tile([128, NT], f32, tag="sdsum", name=f"sds{b}")
            nc.vector.tensor_add(sd_sum[:], sdc_ps[:], mk[:, 1:NT + 1])
            exps = s['exps'] = vec.tile([128, NT], f32, tag="exps",
                                        name=f"exps{b}")
            nc.scalar.activation(exps[:], sd_sum[:], AF.Exp)
            for hf in range(2):
                nc.scalar.activation(ET[:, ts(hf, HHALF)], ST[:, ts(hf, HHALF)],
                                     AF.Exp, bias=s['sqb'][:])
            for dc in range(NT):
                nc.tensor.matmul(rc_ps[:, dc:dc + 1], ET[:, ts(dc, 128)],
                                 ones16[:], start=True, stop=True)
            nc.vector.reciprocal(s['rinv'][:], rc_ps[:])

        def stage_E1(b, fill=(), dcs=tuple(range(NT))):
            s = st[b]
            Ud, Uq16, ET, rinv = s['Ud'], s['Uq16'], s['ET'], s['rinv']
            out2s = {}
            for dc in dcs:
                lhs = ET[:, ts(dc, 128)]
                rdc = rinv[:, dc:dc + 1]
                out2 = outp.tile([128, 2, H], bf16, tag="out2",
                                 name=f"out2_{b}_{dc}")
                out2s[dc] = out2
                a_ps = ps_pp.tile([128, H], f32, tag="pp",
                                  name=f"aps{b}_{dc}")
                for hf in range(2):
                    nc.tensor.matmul(a_ps[:, ts(hf, HHALF)], lhs,
                                     Uq16[:, ts(hf, HHALF)],
                                     start=True, stop=True)
                if AD_ENG[b][dc] == 'A':
                    nc.scalar.mul(out2[:, 0, :], a_ps[:], rdc)
                else:
                    nc.vector.tensor_scalar_mul(out2[:, 0, :], a_ps[:], rdc)
            order = ([dc for dc in dcs if C3_ENG[b][dc] != 'G'] +
                     [dc for dc in dcs if C3_ENG[b][dc] == 'G'])
            for dc in order:
                out2 = out2s[dc]
                eng = nc.gpsimd if C3_ENG[b][dc] == 'G' else nc.vector
                eng.tensor_mul(out2[:, 1, :], out2[:, 0, :], Ud[:, dc, :])
            for n, dc in enumerate(order):
                rows = slice(dc * 128, (dc + 1) * 128)
                nc.sync.dma_start(V_dram[b, rows, H:3 * H], out2s[dc][:])
                for fb, fi in dict(fill).get(n, ()):
                    udsec(fb, fi)

        def cd_en(b):
            s = st[b]
            ET, exps = s['ET'], s['exps']
            EN = s['EN'] = med.tile([128, NT, Q], bf16, tag="EN",
                                    name=f"EN{b}")
            te = ps_sd.tile([128, NT * 128], bf16, tag="te", name=f"te{b}")
            for ec in range(NT):
                nc.tensor.transpose(te[:, ts(ec, 128)],
                                    ET[:, ts(ec, 128)], ident16[:])
            for ec in range(NT):
                if ENP_EVAC[ec] == 'A':
                    nc.scalar.mul(EN[:, ec, :], te[:, ts(ec, 128)],
                                  exps[:, ec:ec + 1])
                else:
                    nc.vector.tensor_scalar_mul(EN[:, ec, :],
                                                te[:, ts(ec, 128)],
                                                exps[:, ec:ec + 1])

        def cd_w(b):
            s = st[b]
            Ud, EN = s['Ud'], s['EN']
            Wb = ps_pp.tile([128, H], f32, tag="pp", name=f"Wb{b}")
            for hf in range(2):
                for et in range(NT):
                    nc.tensor.matmul(Wb[:, ts(hf, HHALF)], EN[:, et, :],
                                     Ud[:, et, ts(hf, HHALF)],
                                     start=(et == 0), stop=(et == NT - 1))
            smB = ps_pp.tile([128, 1], f32, tag="pp", name=f"c2_{b}",
                             padded_shape=[128, 1024])
            for et in range(NT):
                nc.tensor.matmul(smB[:], EN[:, et, :], ones16[:],
                                 start=(et == 0), stop=(et == NT - 1))
            c2inv = vec.tile([128, 1], f32, tag="c2inv")
            nc.vector.reciprocal(c2inv[:], smB[:])
            W = s['W'] = med.tile([128, H], bf16, tag="W", name=f"W{b}")
            nc.vector.tensor_scalar_mul(W[:], Wb[:], c2inv[:])

        def stage_CD(b):
            cd_en(b)
            cd_w(b)

        def stage_E2(b, fill=(), dcs=tuple(range(NT))):
            s = st[b]
            Ud, ET, rinv, W = s['Ud'], s['ET'], s['rinv'], s['W']
            out4s, a4s = {}, {}
            for dc in dcs:
                lhs = ET[:, ts(dc, 128)]
                rdc = rinv[:, dc:dc + 1]
                out4 = outp.tile([128, H], bf16, tag="out4",
                                 name=f"out4_{b}_{dc}")
                out4s[dc] = out4
                r_ps = ps_pp.tile([128, H], f32, tag="pp",
                                  name=f"rps{b}_{dc}")
                for hf in range(2):
                    nc.tensor.matmul(r_ps[:, ts(hf, HHALF)], lhs,
                                     W[:, ts(hf, HHALF)],
                                     start=True, stop=True)
                if C4_ENG[b][dc] == 'D':
                    nc.vector.scalar_tensor_tensor(
                        out4[:], r_ps[:], rdc, Ud[:, dc, :],
                        ALU.mult, ALU.mult)
                else:
                    A4 = outp.tile([128, H], bf16, tag="A4",
                                   name=f"A4_{b}_{dc}")
                    nc.scalar.mul(A4[:], r_ps[:], rdc)
                    a4s[dc] = A4
            nD = 0
            for dc in dcs:
                if C4_ENG[b][dc] == 'D':
                    rows = slice(dc * 128, (dc + 1) * 128)
                    nc.sync.dma_start(V_dram[b, rows, 3 * H:4 * H],
                                      out4s[dc][:])
                    nD += 1
                    for fb, fi in dict(fill).get(nD - 1, ()):
                        udsec(fb, fi)
            order = ([dc for dc in dcs if C4_ENG[b][dc] == 'G'] +
                     [dc for dc in dcs if C4_ENG[b][dc] == 'M'])
            for dc in order:
                eng = nc.gpsimd if C4_ENG[b][dc] == 'G' else nc.vector
                eng.tensor_mul(out4s[dc][:], a4s[dc][:], Ud[:, dc, :])
            for n, dc in enumerate(order):
                rows = slice(dc * 128, (dc + 1) * 128)
                nc.sync.dma_start(V_dram[b, rows, 3 * H:4 * H], out4s[dc][:])
                for fb, fi in dict(fill).get(('m', n), ()):
                    udsec(fb, fi)

        stage_AB(0)
        udsec(0, 0)
        udsec(0, 1)
        nc.sync.dma_start(*st[1]['dma_late'])
        udsec(0, 2)
        stage_E1(0, fill={3: [(0, 3)]})
        stage_AB(1)
        udsec(0, 4)
        cd_en(0)
        stage_E1(1, fill={1: [(0, 5)]}, dcs=(0, 1, 2, 3))
        udsec(0, 6)
        cd_w(0)
        stage_E1(1, fill={1: [(0, 7)]}, dcs=(4, 5, 6, 7))
        udsec(1, 0)
        udsec(1, 1)
        cd_en(1)
        stage_E2(0, fill={1: [(1, 2)]}, dcs=(0, 1, 2, 3))
        udsec(1, 3)
        cd_w(1)
        stage_E2(0, fill={1: [(1, 4), (1, 5)], ('m', 0): [(1, 6), (1, 7)]},
                 dcs=(4, 5, 6, 7))
        stage_E2(1)

    nc.compile()
    return nc


def _get_nc():
    if 'nc' not in _CACHE:
        _CACHE['nc'] = build_nc()
    return _CACHE['nc']


def make_in_maps(inputs):
    import ml_dtypes
    bf16 = ml_dtypes.bfloat16
    U_d = np.asarray(inputs['U_d'], dtype=np.float32).astype(bf16)
    U_q = np.asarray(inputs['U_q'], dtype=np.float32).astype(bf16)
    wc_w = np.asarray(inputs['wc_w'], dtype=np.float32)
    q_mask = np.asarray(inputs['q_mask'], dtype=np.int32)
    d_mask = np.asarray(inputs['d_mask'], dtype=np.int32)
    w_cols = np.ascontiguousarray(
        wc_w.reshape(3, NT, 128).transpose(2, 0, 1))
    qbias = ((q_mask.astype(np.float32) - 1.0) * 30.0)[:, :, None]
    dbias = ((d_mask.astype(np.float32) - 1.0) * 30.0) \
        .reshape(B, NT, 128).transpose(0, 2, 1)
    mbias = np.ascontiguousarray(
        np.concatenate([qbias, dbias], axis=2))
    in_maps = []
    for c in range(NCORES):
        s = slice(c * NB, (c + 1) * NB)
        in_maps.append({
            'U_d': np.ascontiguousarray(U_d[s]),
            'U_q': np.ascontiguousarray(U_q[s]),
            'wc_w': w_cols,
            'd_mask': mbias[s],
        })
    return in_maps


def run(inputs, trace=False, **kw):
    from concourse.bass_utils import run_bass_kernel_spmd
    nc = _get_nc()
    res = run_bass_kernel_spmd(nc, make_in_maps(inputs), list(range(NCORES)),
                               trace=trace, **kw)
    out = np.concatenate(
        [np.asarray(res.results[c]['V']).astype(np.float32)
         for c in range(NCORES)], axis=0)
    return out, res


def kernel(**inputs) -> np.ndarray:
    out, _ = run(inputs, trace=False)
    return out
